# revision 23
# baseline (speedup 1.0000x reference)
"""Neural spline transformer (rational-quadratic spline flow) on 8 trn2 cores.

Data-parallel over the batch dim. Per core:
  x:[256,512] params:[256,97,512] -> y:[256,512], log_det:[256]

Layout strategy: transpose each batch's [97,512] param slab (PE) so partitions
index features; free dim indexes bins. Cumulative knots via a segmented
tensor_tensor_scan; bin search + 6-way gather via a 5-level binary
select tree (is_gt + copy_predicated halving); finals as wide fp32 vector ops.
"""

import numpy as np

import concourse.bacc as baccmod
import concourse.bass as bass
import concourse.mybir as mybir
import concourse.tile as tile
from concourse import masks
from concourse.bass_utils import run_bass_kernel_spmd

F32 = mybir.dt.float32
AL = mybir.AluOpType
AF = mybir.ActivationFunctionType

B, NP, F = 2048, 97, 512
K = 32
NCORES = 8
BC = B // NCORES            # 256 batches per core
NFB = F // 128              # 4 feature blocks
G = 16                      # batches per supergroup
NSG = BC // G               # supergroups per core
W = NFB * G                 # 64 columns (fb-major) per supergroup
RA = 4                      # A-record floats: [ew, eh, es, es1]
RB = 2                      # B-record floats: [C_j, D_j]
AG = K * RA                 # 128 floats per A column
BG = K * RB                 # 64 floats per B column


def build_kernel():
    nc = baccmod.Bacc(None)
    x_in = nc.dram_tensor("x", [BC, F], F32, kind="ExternalInput")
    p_in = nc.dram_tensor("p", [BC, NP, F], F32, kind="ExternalInput")
    x0_in = nc.dram_tensor("x0c", [128, NFB], F32, kind="ExternalInput")
    xf_in = nc.dram_tensor("spanc", [128, 2 * NFB], F32, kind="ExternalInput")
    y_out = nc.dram_tensor("y", [BC, F], F32, kind="ExternalOutput")
    ld_out = nc.dram_tensor("ld", [1, BC], F32, kind="ExternalOutput")

    with tile.TileContext(nc) as tc:
        _body(tc, nc, x_in, p_in, x0_in, xf_in, y_out, ld_out)
    nc.finalize()
    return nc


def _body(tc, nc, x_in, p_in, x0_in, xf_in, y_out, ld_out):
    from contextlib import ExitStack

    with ExitStack() as ctx:
        singles = ctx.enter_context(tc.tile_pool(name="singles", bufs=1))
        ppool = ctx.enter_context(tc.tile_pool(name="pstage", bufs=3))
        tppool = ctx.enter_context(
            tc.tile_pool(name="tpsum", bufs=2, space="PSUM")
        )
        smallps = ctx.enter_context(
            tc.tile_pool(name="smallps", bufs=2, space="PSUM")
        )
        apool = ctx.enter_context(tc.tile_pool(name="apool", bufs=2))
        bpool = ctx.enter_context(tc.tile_pool(name="bpool", bufs=2))
        spool = ctx.enter_context(tc.tile_pool(name="stage", bufs=2))
        ypool = ctx.enter_context(tc.tile_pool(name="ypool", bufs=2))

        ident = singles.tile([128, 128], F32)
        masks.make_identity(nc, ident[:])
        ones_col = singles.tile([128, 1], F32)
        nc.vector.memset(ones_col[:], 1.0)
        # prime PE's clock on ident so transposes carry a single sync wait
        warmps = smallps.tile([1, 1], F32, tag="ldp", name="warmps")
        nc.tensor.matmul(
            warmps[:], ident[0:1, 0:1], ident[0:1, 0:1], start=True, stop=True
        )

        # segmented-scan mask: 0.0 at each column's j=0, else 1.0
        maskc = singles.tile([128, W * K], F32)
        it = singles.tile([128, W * K], mybir.dt.int32)
        nc.gpsimd.iota(
            it[:], pattern=[[0, W], [1, K]], base=0, channel_multiplier=0
        )
        nc.vector.tensor_scalar(
            out=maskc[:], in0=it[:], scalar1=0.5, scalar2=0.0,
            op0=AL.is_gt, op1=AL.bypass,
        )

        # host-prepped per-feature tiles: x0, span=xf-x0, ispan=1/span
        x0sb = singles.tile([128, NFB], F32)
        spanisb = singles.tile([128, 2 * NFB], F32)
        nc.sync.dma_start(out=x0sb[:], in_=x0_in[:])
        nc.sync.dma_start(out=spanisb[:], in_=xf_in[:])
        spansb = spanisb[:, 0:NFB]
        ispansb = spanisb[:, NFB:2 * NFB]
        # prime DVE's clock on the const tiles (keeps later waits at <=2)
        dvewarm = singles.tile([1, 1], F32)
        nc.vector.tensor_tensor(
            out=dvewarm[:], in0=x0sb[0:1, 0:1], in1=spanisb[0:1, 0:1],
            op=AL.add,
        )

        # ---- u = (x - x0) * ispan, transposed to [128f, NFB, BC] ----
        ut = singles.tile([128, NFB, BC], F32)
        for half in range(BC // 128):
            xs = ppool.tile([128, F], F32, tag="xstage")
            nc.sync.dma_start(
                out=xs[:], in_=x_in[half * 128:(half + 1) * 128, :]
            )
            for fb in range(NFB):
                xps = smallps.tile([128, 128], F32, tag="tps")
                nc.tensor.transpose(
                    xps[:], xs[:, fb * 128:(fb + 1) * 128], ident[:]
                )
                # u = (xT - x0) * ispan  (per-partition scalars for this fb)
                nc.vector.tensor_scalar(
                    out=ut[:, fb, half * 128:(half + 1) * 128],
                    in0=xps[:],
                    scalar1=x0sb[:, fb:fb + 1],
                    scalar2=ispansb[:, fb:fb + 1],
                    op0=AL.subtract,
                    op1=AL.mult,
                )

        ldsb = singles.tile([1, BC], F32)

        for sg in range(NSG):
            b0 = sg * G
            at = apool.tile([128, 4 + W * AG], F32)
            bt = bpool.tile([128, W * BG], F32)
            av = at[:, 4:].rearrange("p (c f) -> p c f", f=AG)
            bv = bt[:].rearrange("p (c f) -> p c f", f=BG)
            nc.vector.memset(at[:, 0:4], 0.0)

            GH = G // 2  # half-supergroup chunk (PSUM budget)
            for fb in range(NFB):
                for hf in range(2):
                    g0 = hf * GH
                    ps = ppool.tile([97, GH * 128], F32, tag="pstage")
                    for g in range(GH):
                        nc.sync.dma_start(
                            out=ps[:, g * 128:(g + 1) * 128],
                            in_=p_in[
                                b0 + g0 + g, :, fb * 128:(fb + 1) * 128
                            ],
                        )
                    pp = tppool.tile([128, GH * 128], F32, tag="ppsum")
                    ppv = pp[:].rearrange("p (g c) -> p g c", g=GH)
                    for g in range(GH):
                        nc.tensor.transpose(
                            ppv[:, g, 0:97],
                            ps[:, g * 128:g * 128 + 128][0:97, :],
                            ident[0:97, 0:97],
                        )
                    # exp of rows 0..95 -> slots 0..2 (ew, eh, es)
                    a0 = 4 + (fb * G + g0) * AG
                    acols = at[:, a0:a0 + GH * AG]
                    nc.scalar.activation(
                        out=acols.rearrange(
                            "p (g j s) -> p g s j", j=K, s=RA
                        )[:, :, 0:3, :],
                        in_=ppv[:, :, 0:96].rearrange(
                            "p g (s j) -> p g s j", s=3
                        ),
                        func=AF.Exp,
                    )
                    # exp of slope rows 65..96 -> slot 3
                    # (es1[j] = exp(slope[j+1]))
                    nc.scalar.activation(
                        out=acols.rearrange(
                            "p (g j s) -> p g s j", j=K, s=RA
                        )[:, :, 3, :],
                        in_=ppv[:, :, 65:97],
                        func=AF.Exp,
                    )

            # ---- segmented exclusive cumsums into B records ----
            # state = (ew[j-1] + state) * mask[j];  mask kills at j=0
            d0w = at[:, 0:W * AG].rearrange("p (m s) -> p m s", s=RA)[:, :, 0]
            d0h = at[:, 1:1 + W * AG].rearrange("p (m s) -> p m s", s=RA)[
                :, :, 0
            ]
            outw = bt[:].rearrange("p (m s) -> p m s", s=RB)[:, :, 0]
            outh = bt[:].rearrange("p (m s) -> p m s", s=RB)[:, :, 1]
            nc.vector.tensor_tensor_scan(
                out=outw, data0=d0w, data1=maskc[:], initial=0.0,
                op0=AL.add, op1=AL.mult,
            )
            nc.vector.tensor_tensor_scan(
                out=outh, data0=d0h, data1=maskc[:], initial=0.0,
                op0=AL.add, op1=AL.mult,
            )

            # ---- per-column scalars ----
            sw = spool.tile([128, W], F32, tag="sw")
            sh = spool.tile([128, W], F32, tag="sh")
            tt = spool.tile([128, W], F32, tag="tt")
            nc.vector.tensor_tensor(
                out=sw[:], in0=bv[:, :, BG - 2], in1=av[:, :, AG - 4],
                op=AL.add,
            )
            nc.vector.tensor_tensor(
                out=sh[:], in0=bv[:, :, BG - 1], in1=av[:, :, AG - 3],
                op=AL.add,
            )
            nc.vector.tensor_tensor(
                out=tt[:].rearrange("p (fb g) -> p fb g", fb=NFB),
                in0=ut[:, :, b0:b0 + G],
                in1=sw[:].rearrange("p (fb g) -> p fb g", fb=NFB),
                op=AL.mult,
            )

            # ---- binary-search select tree (in place halving) ----
            bits = spool.tile([128, 5 * W], mybir.dt.uint8, tag="bits")
            for lv in range(5):
                h = 16 >> lv  # half-window in records
                bcol = bits[:, lv * W:(lv + 1) * W]
                nc.vector.tensor_tensor(
                    out=bcol, in0=tt[:], in1=bv[:, :, h * RB], op=AL.is_gt
                )
                nc.vector.copy_predicated(
                    out=av[:, :, 0:h * RA],
                    mask=bcol.unsqueeze(2).broadcast_to([128, W, h * RA]),
                    data=av[:, :, h * RA:2 * h * RA],
                )
                nc.vector.copy_predicated(
                    out=bv[:, :, 0:h * RB],
                    mask=bcol.unsqueeze(2).broadcast_to([128, W, h * RB]),
                    data=bv[:, :, h * RB:2 * h * RB],
                )

            ew = av[:, :, 0]
            eh = av[:, :, 1]
            es0 = av[:, :, 2]
            es1 = av[:, :, 3]
            ci = bv[:, :, 0]
            di = bv[:, :, 1]

            # ---- finals ----
            def ftile(tag):
                return spool.tile([128, W], F32, tag=tag, name=tag)

            def tt_(out, a, b_, op):
                nc.vector.tensor_tensor(out=out, in0=a, in1=b_, op=op)

            ic = ftile("ic")
            scr = ftile("scr")
            nc.vector.reciprocal_approx_accurate(
                out=ic[:], in_=ew, scratch=scr[:]
            )
            ish = ftile("ish")
            nc.vector.reciprocal_approx_accurate(
                out=ish[:], in_=sh[:], scratch=scr[:]
            )
            d0 = ftile("d0")
            nc.scalar.activation(out=d0[:], in_=es0, func=AF.Ln, bias=1.0)
            d1 = ftile("d1")
            nc.scalar.activation(out=d1[:], in_=es1, func=AF.Ln, bias=1.0)

            eps = ftile("eps")
            tt_(eps[:], tt[:], ci, AL.subtract)
            tt_(eps[:], eps[:], ic[:], AL.mult)
            onem = ftile("onem")
            nc.vector.tensor_scalar(
                out=onem[:], in0=eps[:], scalar1=-1.0, scalar2=1.0,
                op0=AL.mult, op1=AL.add,
            )
            e1m = ftile("e1m")
            tt_(e1m[:], eps[:], onem[:], AL.mult)
            e2 = ftile("e2")
            tt_(e2[:], eps[:], eps[:], AL.mult)
            om2 = ftile("om2")
            tt_(om2[:], onem[:], onem[:], AL.mult)

            s_ = ftile("s_")
            tt_(s_[:], eh, sw[:], AL.mult)
            tt_(s_[:], s_[:], ic[:], AL.mult)
            tt_(s_[:], s_[:], ish[:], AL.mult)

            den = ftile("den")
            tt_(den[:], d1[:], d0[:], AL.add)
            nc.vector.tensor_scalar(
                out=scr[:], in0=s_[:], scalar1=-2.0, scalar2=0.0,
                op0=AL.mult, op1=AL.bypass,
            )
            tt_(den[:], den[:], scr[:], AL.add)
            tt_(den[:], den[:], e1m[:], AL.mult)
            tt_(den[:], den[:], s_[:], AL.add)
            iden = ftile("iden")
            nc.vector.reciprocal_approx_accurate(
                out=iden[:], in_=den[:], scratch=scr[:]
            )

            ny = ftile("ny")
            tt_(ny[:], s_[:], e2[:], AL.mult)
            nyb = ftile("nyb")
            tt_(nyb[:], d0[:], e1m[:], AL.mult)
            tt_(ny[:], ny[:], nyb[:], AL.add)
            hn = ftile("hn")
            tt_(hn[:], eh, ish[:], AL.mult)
            tt_(ny[:], ny[:], hn[:], AL.mult)
            tt_(ny[:], ny[:], iden[:], AL.mult)
            dis = ftile("dis")
            tt_(dis[:], di, ish[:], AL.mult)
            tt_(ny[:], ny[:], dis[:], AL.add)
            # y = x0 + span * ny   (x0/span vary per (partition, fb))
            yt = ypool.tile([128, W], F32, tag="yt")
            ytv = yt[:].rearrange("p (fb g) -> p fb g", fb=NFB)
            nyv = ny[:].rearrange("p (fb g) -> p fb g", fb=NFB)
            nc.vector.tensor_tensor(
                out=ytv, in0=nyv,
                in1=spansb.unsqueeze(2).broadcast_to([128, NFB, G]),
                op=AL.mult,
            )
            nc.vector.tensor_tensor(
                out=ytv, in0=ytv,
                in1=x0sb[:].unsqueeze(2).broadcast_to([128, NFB, G]),
                op=AL.add,
            )

            # dy/dx = (s/den)^2 * (d1*e2 + 2*s*e1m + d0*(1-eps)^2)
            n2 = ftile("n2")
            tt_(n2[:], d1[:], e2[:], AL.mult)
            n2b = ftile("n2b")
            tt_(n2b[:], s_[:], e1m[:], AL.mult)
            nc.vector.tensor_scalar(
                out=n2b[:], in0=n2b[:], scalar1=2.0, scalar2=0.0,
                op0=AL.mult, op1=AL.bypass,
            )
            tt_(n2[:], n2[:], n2b[:], AL.add)
            tt_(n2b[:], d0[:], om2[:], AL.mult)
            tt_(n2[:], n2[:], n2b[:], AL.add)
            p_ = ftile("p_")
            tt_(p_[:], s_[:], iden[:], AL.mult)
            tt_(p_[:], p_[:], p_[:], AL.mult)
            tt_(n2[:], n2[:], p_[:], AL.mult)
            ldy = ftile("ldy")
            nc.scalar.activation(out=ldy[:], in_=n2[:], func=AF.Ln)

            # log_det: sum over features = 128 partitions x NFB columns
            ldp = smallps.tile([1, G], F32, tag="ldp")
            for fb in range(NFB):
                nc.tensor.matmul(
                    ldp[:], ones_col[:], ldy[:, fb * G:(fb + 1) * G],
                    start=(fb == 0), stop=(fb == NFB - 1),
                )
            nc.vector.tensor_copy(ldsb[:, b0:b0 + G], ldp[:])

            # y back to [b, f] layout and out
            yps = smallps.tile([W, 128], F32, tag="tps", name="yps")
            nc.tensor.transpose(yps[:], yt[:], ident[:])
            ysb = ypool.tile([W, 128], F32, tag="ysb")
            nc.vector.tensor_copy(ysb[:], yps[:])
            nc.sync.dma_start(
                out=y_out[b0:b0 + G, :].rearrange(
                    "b (fb f) -> fb b f", fb=NFB
                ),
                in_=ysb[:],
            )

        nc.sync.dma_start(out=ld_out[:], in_=ldsb[:])


_CACHE = {}


def _prep_consts(x0, xf):
    x0 = np.asarray(x0, np.float32)
    xf = np.asarray(xf, np.float32)
    span = xf - x0
    x0c = np.ascontiguousarray(x0.reshape(NFB, 128).T)
    spanc = np.ascontiguousarray(
        np.concatenate(
            [span.reshape(NFB, 128).T, (1.0 / span).reshape(NFB, 128).T],
            axis=1,
        )
    )
    return x0c, spanc


def kernel(x, parameters, x0, xf):
    if "nc" not in _CACHE:
        _CACHE["nc"] = build_kernel()
    nc = _CACHE["nc"]
    x0c, spanc = _prep_consts(x0, xf)
    in_maps = []
    for i in range(NCORES):
        in_maps.append(
            {
                "x": np.ascontiguousarray(x[i * BC:(i + 1) * BC]),
                "p": np.ascontiguousarray(parameters[i * BC:(i + 1) * BC]),
                "x0c": x0c,
                "spanc": spanc,
            }
        )
    res = run_bass_kernel_spmd(nc, in_maps, list(range(NCORES)))
    y = np.concatenate([r["y"] for r in res.results], axis=0)
    ld = np.concatenate([r["ld"].reshape(BC) for r in res.results], axis=0)
    return y, ld


# revision 25
# speedup vs baseline: 1.1614x; 1.1614x over previous
"""Neural spline transformer (rational-quadratic spline flow) on 8 trn2 cores.

Data-parallel over the batch dim. Per core:
  x:[256,512] params:[256,97,512] -> y:[256,512], log_det:[256]

Layout strategy: transpose each batch's [97,512] param slab (PE) so partitions
index features; free dim indexes bins. Cumulative knots via a segmented
tensor_tensor_scan; bin search + 6-way gather via a 5-level binary
select tree (is_gt + copy_predicated halving); finals as wide fp32 vector ops.
"""

import numpy as np

import concourse.bacc as baccmod
import concourse.bass as bass
import concourse.mybir as mybir
import concourse.tile as tile
from concourse import masks
from concourse.bass_utils import run_bass_kernel_spmd

F32 = mybir.dt.float32
AL = mybir.AluOpType
AF = mybir.ActivationFunctionType

B, NP, F = 2048, 97, 512
K = 32
NCORES = 8
BC = B // NCORES            # 256 batches per core
NFB = F // 128              # 4 feature blocks
G = 16                      # batches per supergroup
NSG = BC // G               # supergroups per core
W = NFB * G                 # 64 columns (fb-major) per supergroup
RA = 4                      # A-record floats: [ew, eh, es, es1]
RB = 2                      # B-record floats: [C_j, D_j]
AG = K * RA                 # 128 floats per A column
BG = K * RB                 # 64 floats per B column


def build_kernel():
    nc = baccmod.Bacc(None)
    x_in = nc.dram_tensor("x", [BC, F], F32, kind="ExternalInput")
    p_in = nc.dram_tensor("p", [BC, NP, F], F32, kind="ExternalInput")
    x0_in = nc.dram_tensor("x0c", [128, NFB], F32, kind="ExternalInput")
    xf_in = nc.dram_tensor("spanc", [128, 2 * NFB], F32, kind="ExternalInput")
    y_out = nc.dram_tensor("y", [BC, F], F32, kind="ExternalOutput")
    ld_out = nc.dram_tensor("ld", [1, BC], F32, kind="ExternalOutput")

    with tile.TileContext(nc) as tc:
        _body(tc, nc, x_in, p_in, x0_in, xf_in, y_out, ld_out)
    nc.finalize()
    return nc


def _body(tc, nc, x_in, p_in, x0_in, xf_in, y_out, ld_out):
    from contextlib import ExitStack

    with ExitStack() as ctx:
        singles = ctx.enter_context(tc.tile_pool(name="singles", bufs=1))
        ppool = ctx.enter_context(tc.tile_pool(name="pstage", bufs=2))
        tppool = ctx.enter_context(
            tc.tile_pool(name="tpsum", bufs=2, space="PSUM")
        )
        smallps = ctx.enter_context(
            tc.tile_pool(name="smallps", bufs=2, space="PSUM")
        )
        apool = ctx.enter_context(tc.tile_pool(name="apool", bufs=2))
        bpool = ctx.enter_context(tc.tile_pool(name="bpool", bufs=2))
        spool = ctx.enter_context(tc.tile_pool(name="stage", bufs=2))
        ypool = ctx.enter_context(tc.tile_pool(name="ypool", bufs=2))

        ident = singles.tile([128, 128], F32)
        masks.make_identity(nc, ident[:])
        ones_col = singles.tile([128, 1], F32)
        nc.vector.memset(ones_col[:], 1.0)
        # prime PE's clock on ident so transposes carry a single sync wait
        warmps = smallps.tile([1, 1], F32, tag="ldp", name="warmps")
        nc.tensor.matmul(
            warmps[:], ident[0:1, 0:1], ident[0:1, 0:1], start=True, stop=True
        )

        # segmented-scan mask: 0.0 at each column's j=0, else 1.0
        maskc = singles.tile([128, W * K], F32)
        it = singles.tile([128, W * K], mybir.dt.int32)
        nc.gpsimd.iota(
            it[:], pattern=[[0, W], [1, K]], base=0, channel_multiplier=0
        )
        nc.vector.tensor_scalar(
            out=maskc[:], in0=it[:], scalar1=0.5, scalar2=0.0,
            op0=AL.is_gt, op1=AL.bypass,
        )

        # host-prepped per-feature tiles: x0, span=xf-x0, ispan=1/span
        x0sb = singles.tile([128, NFB], F32)
        spanisb = singles.tile([128, 2 * NFB], F32)
        nc.sync.dma_start(out=x0sb[:], in_=x0_in[:])
        nc.sync.dma_start(out=spanisb[:], in_=xf_in[:])
        spansb = spanisb[:, 0:NFB]
        ispansb = spanisb[:, NFB:2 * NFB]
        # prime DVE's clock on the const tiles (keeps later waits at <=2)
        dvewarm = singles.tile([1, 1], F32)
        nc.vector.tensor_tensor(
            out=dvewarm[:], in0=x0sb[0:1, 0:1], in1=spanisb[0:1, 0:1],
            op=AL.add,
        )

        # ---- u = (x - x0) * ispan, transposed to [128f, NFB, BC] ----
        ut = singles.tile([128, NFB, BC], F32)
        for half in range(BC // 128):
            xs = ppool.tile([128, F], F32, tag="xstage")
            nc.sync.dma_start(
                out=xs[:], in_=x_in[half * 128:(half + 1) * 128, :]
            )
            for fb in range(NFB):
                xps = smallps.tile([128, 128], F32, tag="tps")
                nc.tensor.transpose(
                    xps[:], xs[:, fb * 128:(fb + 1) * 128], ident[:]
                )
                # u = (xT - x0) * ispan  (per-partition scalars for this fb)
                nc.vector.tensor_scalar(
                    out=ut[:, fb, half * 128:(half + 1) * 128],
                    in0=xps[:],
                    scalar1=x0sb[:, fb:fb + 1],
                    scalar2=ispansb[:, fb:fb + 1],
                    op0=AL.subtract,
                    op1=AL.mult,
                )

        ldsb = singles.tile([1, BC], F32)

        for sg in range(NSG):
            b0 = sg * G
            at = apool.tile([128, 4 + W * AG], F32)
            bt = bpool.tile([128, W * BG], F32)
            av = at[:, 4:].rearrange("p (c f) -> p c f", f=AG)
            bv = bt[:].rearrange("p (c f) -> p c f", f=BG)
            nc.vector.memset(at[:, 0:4], 0.0)

            GH = G // 2  # half-supergroup chunk (PSUM budget)
            for hf in range(2):
                g0 = hf * GH
                # one big (~1.5 MiB) DMA per half-supergroup: fans out
                # across SDMA engines instead of serializing on one queue
                ps = ppool.tile([97, GH * F], F32, tag="pstage")
                dma_eng = nc.sync if hf == 0 else nc.scalar
                dma_eng.dma_start(
                    out=ps[:].rearrange("p (g f) -> p g f", g=GH),
                    in_=p_in[b0 + g0:b0 + g0 + GH, :, :].transpose(
                        [1, 0, 2]
                    ),
                )
                for fb in range(NFB):
                    pp = tppool.tile([128, GH * 128], F32, tag="ppsum")
                    ppv = pp[:].rearrange("p (g c) -> p g c", g=GH)
                    for g in range(GH):
                        f0 = g * F + fb * 128
                        nc.tensor.transpose(
                            ppv[:, g, 0:97],
                            ps[:, f0:f0 + 128][0:97, :],
                            ident[0:97, 0:97],
                        )
                    # exp of rows 0..95 -> slots 0..2 (ew, eh, es)
                    a0 = 4 + (fb * G + g0) * AG
                    acols = at[:, a0:a0 + GH * AG]
                    nc.scalar.activation(
                        out=acols.rearrange(
                            "p (g j s) -> p g s j", j=K, s=RA
                        )[:, :, 0:3, :],
                        in_=ppv[:, :, 0:96].rearrange(
                            "p g (s j) -> p g s j", s=3
                        ),
                        func=AF.Exp,
                    )
                    # exp of slope rows 65..96 -> slot 3
                    # (es1[j] = exp(slope[j+1]))
                    nc.scalar.activation(
                        out=acols.rearrange(
                            "p (g j s) -> p g s j", j=K, s=RA
                        )[:, :, 3, :],
                        in_=ppv[:, :, 65:97],
                        func=AF.Exp,
                    )

            # ---- segmented exclusive cumsums into B records ----
            # state = (ew[j-1] + state) * mask[j];  mask kills at j=0
            d0w = at[:, 0:W * AG].rearrange("p (m s) -> p m s", s=RA)[:, :, 0]
            d0h = at[:, 1:1 + W * AG].rearrange("p (m s) -> p m s", s=RA)[
                :, :, 0
            ]
            outw = bt[:].rearrange("p (m s) -> p m s", s=RB)[:, :, 0]
            outh = bt[:].rearrange("p (m s) -> p m s", s=RB)[:, :, 1]
            nc.vector.tensor_tensor_scan(
                out=outw, data0=d0w, data1=maskc[:], initial=0.0,
                op0=AL.add, op1=AL.mult,
            )
            nc.vector.tensor_tensor_scan(
                out=outh, data0=d0h, data1=maskc[:], initial=0.0,
                op0=AL.add, op1=AL.mult,
            )

            # ---- per-column scalars ----
            sw = spool.tile([128, W], F32, tag="sw")
            sh = spool.tile([128, W], F32, tag="sh")
            tt = spool.tile([128, W], F32, tag="tt")
            nc.vector.tensor_tensor(
                out=sw[:], in0=bv[:, :, BG - 2], in1=av[:, :, AG - 4],
                op=AL.add,
            )
            nc.vector.tensor_tensor(
                out=sh[:], in0=bv[:, :, BG - 1], in1=av[:, :, AG - 3],
                op=AL.add,
            )
            nc.vector.tensor_tensor(
                out=tt[:].rearrange("p (fb g) -> p fb g", fb=NFB),
                in0=ut[:, :, b0:b0 + G],
                in1=sw[:].rearrange("p (fb g) -> p fb g", fb=NFB),
                op=AL.mult,
            )

            # ---- binary-search select tree (in place halving) ----
            bits = spool.tile([128, 5 * W], mybir.dt.uint8, tag="bits")
            for lv in range(5):
                h = 16 >> lv  # half-window in records
                bcol = bits[:, lv * W:(lv + 1) * W]
                nc.vector.tensor_tensor(
                    out=bcol, in0=tt[:], in1=bv[:, :, h * RB], op=AL.is_gt
                )
                nc.vector.copy_predicated(
                    out=av[:, :, 0:h * RA],
                    mask=bcol.unsqueeze(2).broadcast_to([128, W, h * RA]),
                    data=av[:, :, h * RA:2 * h * RA],
                )
                nc.vector.copy_predicated(
                    out=bv[:, :, 0:h * RB],
                    mask=bcol.unsqueeze(2).broadcast_to([128, W, h * RB]),
                    data=bv[:, :, h * RB:2 * h * RB],
                )

            ew = av[:, :, 0]
            eh = av[:, :, 1]
            es0 = av[:, :, 2]
            es1 = av[:, :, 3]
            ci = bv[:, :, 0]
            di = bv[:, :, 1]

            # ---- finals ----
            def ftile(tag):
                return spool.tile([128, W], F32, tag=tag, name=tag)

            def tt_(out, a, b_, op):
                nc.vector.tensor_tensor(out=out, in0=a, in1=b_, op=op)

            ic = ftile("ic")
            scr = ftile("scr")
            nc.vector.reciprocal_approx_accurate(
                out=ic[:], in_=ew, scratch=scr[:]
            )
            ish = ftile("ish")
            nc.vector.reciprocal_approx_accurate(
                out=ish[:], in_=sh[:], scratch=scr[:]
            )
            d0 = ftile("d0")
            nc.scalar.activation(out=d0[:], in_=es0, func=AF.Ln, bias=1.0)
            d1 = ftile("d1")
            nc.scalar.activation(out=d1[:], in_=es1, func=AF.Ln, bias=1.0)

            eps = ftile("eps")
            tt_(eps[:], tt[:], ci, AL.subtract)
            tt_(eps[:], eps[:], ic[:], AL.mult)
            onem = ftile("onem")
            nc.vector.tensor_scalar(
                out=onem[:], in0=eps[:], scalar1=-1.0, scalar2=1.0,
                op0=AL.mult, op1=AL.add,
            )
            e1m = ftile("e1m")
            tt_(e1m[:], eps[:], onem[:], AL.mult)
            e2 = ftile("e2")
            tt_(e2[:], eps[:], eps[:], AL.mult)
            om2 = ftile("om2")
            tt_(om2[:], onem[:], onem[:], AL.mult)

            s_ = ftile("s_")
            tt_(s_[:], eh, sw[:], AL.mult)
            tt_(s_[:], s_[:], ic[:], AL.mult)
            tt_(s_[:], s_[:], ish[:], AL.mult)

            den = ftile("den")
            tt_(den[:], d1[:], d0[:], AL.add)
            nc.vector.tensor_scalar(
                out=scr[:], in0=s_[:], scalar1=-2.0, scalar2=0.0,
                op0=AL.mult, op1=AL.bypass,
            )
            tt_(den[:], den[:], scr[:], AL.add)
            tt_(den[:], den[:], e1m[:], AL.mult)
            tt_(den[:], den[:], s_[:], AL.add)
            iden = ftile("iden")
            nc.vector.reciprocal_approx_accurate(
                out=iden[:], in_=den[:], scratch=scr[:]
            )

            ny = ftile("ny")
            tt_(ny[:], s_[:], e2[:], AL.mult)
            nyb = ftile("nyb")
            tt_(nyb[:], d0[:], e1m[:], AL.mult)
            tt_(ny[:], ny[:], nyb[:], AL.add)
            hn = ftile("hn")
            tt_(hn[:], eh, ish[:], AL.mult)
            tt_(ny[:], ny[:], hn[:], AL.mult)
            tt_(ny[:], ny[:], iden[:], AL.mult)
            dis = ftile("dis")
            tt_(dis[:], di, ish[:], AL.mult)
            tt_(ny[:], ny[:], dis[:], AL.add)
            # y = x0 + span * ny   (x0/span vary per (partition, fb))
            yt = ypool.tile([128, W], F32, tag="yt")
            ytv = yt[:].rearrange("p (fb g) -> p fb g", fb=NFB)
            nyv = ny[:].rearrange("p (fb g) -> p fb g", fb=NFB)
            nc.vector.tensor_tensor(
                out=ytv, in0=nyv,
                in1=spansb.unsqueeze(2).broadcast_to([128, NFB, G]),
                op=AL.mult,
            )
            nc.vector.tensor_tensor(
                out=ytv, in0=ytv,
                in1=x0sb[:].unsqueeze(2).broadcast_to([128, NFB, G]),
                op=AL.add,
            )

            # dy/dx = (s/den)^2 * (d1*e2 + 2*s*e1m + d0*(1-eps)^2)
            n2 = ftile("n2")
            tt_(n2[:], d1[:], e2[:], AL.mult)
            n2b = ftile("n2b")
            tt_(n2b[:], s_[:], e1m[:], AL.mult)
            nc.vector.tensor_scalar(
                out=n2b[:], in0=n2b[:], scalar1=2.0, scalar2=0.0,
                op0=AL.mult, op1=AL.bypass,
            )
            tt_(n2[:], n2[:], n2b[:], AL.add)
            tt_(n2b[:], d0[:], om2[:], AL.mult)
            tt_(n2[:], n2[:], n2b[:], AL.add)
            p_ = ftile("p_")
            tt_(p_[:], s_[:], iden[:], AL.mult)
            tt_(p_[:], p_[:], p_[:], AL.mult)
            tt_(n2[:], n2[:], p_[:], AL.mult)
            ldy = ftile("ldy")
            nc.scalar.activation(out=ldy[:], in_=n2[:], func=AF.Ln)

            # log_det: sum over features = 128 partitions x NFB columns
            ldp = smallps.tile([1, G], F32, tag="ldp")
            for fb in range(NFB):
                nc.tensor.matmul(
                    ldp[:], ones_col[:], ldy[:, fb * G:(fb + 1) * G],
                    start=(fb == 0), stop=(fb == NFB - 1),
                )
            nc.vector.tensor_copy(ldsb[:, b0:b0 + G], ldp[:])

            # y back to [b, f] layout and out
            yps = smallps.tile([W, 128], F32, tag="tps", name="yps")
            nc.tensor.transpose(yps[:], yt[:], ident[:])
            ysb = ypool.tile([W, 128], F32, tag="ysb")
            nc.vector.tensor_copy(ysb[:], yps[:])
            nc.sync.dma_start(
                out=y_out[b0:b0 + G, :].rearrange(
                    "b (fb f) -> fb b f", fb=NFB
                ),
                in_=ysb[:],
            )

        nc.sync.dma_start(out=ld_out[:], in_=ldsb[:])


_CACHE = {}


def _prep_consts(x0, xf):
    x0 = np.asarray(x0, np.float32)
    xf = np.asarray(xf, np.float32)
    span = xf - x0
    x0c = np.ascontiguousarray(x0.reshape(NFB, 128).T)
    spanc = np.ascontiguousarray(
        np.concatenate(
            [span.reshape(NFB, 128).T, (1.0 / span).reshape(NFB, 128).T],
            axis=1,
        )
    )
    return x0c, spanc


def kernel(x, parameters, x0, xf):
    if "nc" not in _CACHE:
        _CACHE["nc"] = build_kernel()
    nc = _CACHE["nc"]
    x0c, spanc = _prep_consts(x0, xf)
    in_maps = []
    for i in range(NCORES):
        in_maps.append(
            {
                "x": np.ascontiguousarray(x[i * BC:(i + 1) * BC]),
                "p": np.ascontiguousarray(parameters[i * BC:(i + 1) * BC]),
                "x0c": x0c,
                "spanc": spanc,
            }
        )
    res = run_bass_kernel_spmd(nc, in_maps, list(range(NCORES)))
    y = np.concatenate([r["y"] for r in res.results], axis=0)
    ld = np.concatenate([r["ld"].reshape(BC) for r in res.results], axis=0)
    return y, ld
